# revision 18
# baseline (speedup 1.0000x reference)
"""Trainium2 Bass kernel for a 2-layer dense GCN block:

    z = x.reshape(B, N, F)                     # B=4, N=8192, F=64
    for i in range(2):
        z = relu((A @ z) @ W_i)                # A: [N, N] dense
    return z

Strategy (8 NeuronCores, SPMD):
  * Shard the output rows (m) of A @ Z across cores: core j owns rows
    [1024*j, 1024*(j+1)).  On the host we hand core j the matching
    column-slice of A^T (so the contraction dim n lands on SBUF
    partitions with a natural, contiguous DMA) cast to bf16 (16 MB —
    stays resident in SBUF for BOTH layers, so A is read from HBM once).
  * Z is handled as a [n, c] matrix with c = b*F + f (256 columns).
    Layer matmuls compute H^T[c, m] = sum_n Z[n, c] * A^T[n, m] on the
    tensor engine (lhsT = Z tile, rhs = A^T tile, fp32 PSUM accum).
  * The per-layer weight apply uses a block-diagonal weight tile
    diag(W_i, W_i) so one K=128 matmul per c-half computes
    Z[m, c] = H^T.T @ diag(W, W) for two batches at once (all operands
    at base partition 0 — base-64 matmuls crash the runtime here).
  * Between layers, an AllGather exchanges each core's 1024-row slice of
    Z1 so every core has the full [8192, 256] Z1.
  * bf16 operands / fp32 accumulation throughout (measured ~0.5% rel-l2
    vs the fp32 reference).  Final output is fp32.
"""

import numpy as np
import ml_dtypes

import concourse.mybir as mybir
import concourse.tile as tile
from concourse import bacc
from concourse.bass_utils import run_bass_kernel_spmd

BF16 = ml_dtypes.bfloat16

NCORES = 8
B, N, F, L = 4, 8192, 64, 2
C = B * F                      # 256 columns of the Z matrix
M_CORE = N // NCORES           # 1024 output rows per core
NT = N // 128                  # 64 contraction tiles of 128
MT = M_CORE // 128             # 8 output-row tiles of 128 per core
KCH = 8                        # big DMA chunks for the resident A^T shard
TPC = NT // KCH                # 8 n-tiles per chunk

_CACHED = {}


def _build_program():
    nc = bacc.Bacc("TRN2", target_bir_lowering=False, debug=False,
                   num_devices=NCORES)
    dt = mybir.dt

    at_d = nc.dram_tensor("at", [N, M_CORE], dt.bfloat16, kind="ExternalInput")
    z0_d = nc.dram_tensor("z0", [N, C], dt.bfloat16, kind="ExternalInput")
    w_d = nc.dram_tensor("w", [128, 2 * 128], dt.bfloat16, kind="ExternalInput")
    out_d = nc.dram_tensor("out", [M_CORE, C], dt.float32, kind="ExternalOutput")

    z1_loc = nc.dram_tensor("z1_loc", [M_CORE, C], dt.bfloat16)
    z1_full = nc.dram_tensor("z1_full", [N, C], dt.bfloat16)

    # DRAM views with the n-tile index split out:  [chunk, p, local tile, cols]
    at_v = at_d.ap().rearrange("(k t p) m -> k p t m", k=KCH, p=128)
    z0_v = z0_d.ap().rearrange("(k t p) c -> k p t c", k=KCH, p=128)
    z1f_v = z1_full.ap().rearrange("(k t p) c -> k p t c", k=KCH, p=128)

    with tile.TileContext(nc) as tc:
        with tc.tile_pool(name="a_res", bufs=1) as a_pool, \
             tc.tile_pool(name="z_res", bufs=1) as z_pool, \
             tc.tile_pool(name="wk", bufs=1) as w_pool, \
             tc.tile_pool(name="ht", bufs=1, space="PSUM") as psh_pool, \
             tc.tile_pool(name="pz", bufs=2, space="PSUM") as psz_pool, \
             tc.tile_pool(name="hsb", bufs=2) as hsb_pool, \
             tc.tile_pool(name="zout", bufs=4) as zout_pool:

            w_sb = w_pool.tile([128, 2 * 128], dt.bfloat16, tag="w")
            nc.scalar.dma_start(out=w_sb[:], in_=w_d[:])

            # Resident A^T shard (16 MB bf16) + Z tiles, loaded in KCH big
            # chunks, interleaved so layer-1 matmuls can start early.
            at_sb = [a_pool.tile([128, TPC * M_CORE], dt.bfloat16,
                                 tag=f"at{k}", name=f"at_sb{k}")
                     for k in range(KCH)]
            z_sb = [z_pool.tile([128, TPC * C], dt.bfloat16,
                                tag=f"z{k}", name=f"z_sb{k}")
                    for k in range(KCH)]

            def free3(tile_ap, inner):
                return tile_ap[:].rearrange("p (t i) -> p t i", i=inner)

            for k in range(KCH):
                nc.scalar.dma_start(out=free3(z_sb[k], C), in_=z0_v[k])
                nc.sync.dma_start(out=free3(at_sb[k], M_CORE), in_=at_v[k])

            def z_tile(t, ch):
                """lhsT: Z[n-tile t, c-half ch] -> [128, 128] bf16."""
                k, tt = divmod(t, TPC)
                return z_sb[k][:, tt * C + ch * 128: tt * C + ch * 128 + 128]

            def at_tile(t, mh):
                """rhs: A^T[n-tile t, m-half mh] -> [128, 512] bf16."""
                k, tt = divmod(t, TPC)
                return at_sb[k][:, tt * M_CORE + mh * 512: tt * M_CORE + mh * 512 + 512]

            def layer(li, out_fp32):
                # --- big matmul: H^T[c, m] accumulated over 64 n-tiles ---
                h_ps = {}
                for ch in range(2):
                    for mh in range(2):
                        h_ps[ch, mh] = psh_pool.tile([128, 512], dt.float32,
                                                     tag=f"hps{ch}{mh}",
                                                     name=f"hps_{li}_{ch}{mh}")
                for ch in range(2):
                    for t in range(NT):
                        for mh in range(2):
                            nc.tensor.matmul(
                                h_ps[ch, mh][:],
                                z_tile(t, ch),
                                at_tile(t, mh),
                                start=(t == 0),
                                stop=(t == NT - 1),
                            )
                # --- PSUM -> SBUF (cast bf16) ---
                h_sb = [hsb_pool.tile([128, M_CORE], dt.bfloat16,
                                      tag=f"h{ch}", name=f"h_sb_{li}_{ch}")
                        for ch in range(2)]
                for ch in range(2):
                    for mh in range(2):
                        nc.vector.tensor_copy(
                            h_sb[ch][:, mh * 512:(mh + 1) * 512],
                            h_ps[ch, mh][:],
                        )
                # --- weight apply + relu: Z[m, c], one matmul per c-half
                # (lhsT = H^T chunk [c=128, m=128], rhs = diag(W, W)) ---
                res = []
                for i in range(MT):
                    z_ps = psz_pool.tile([128, C], dt.float32, tag="zps",
                                         name=f"z_ps_{li}_{i}")
                    for ch in range(2):
                        nc.tensor.matmul(
                            z_ps[:, ch * 128:(ch + 1) * 128],
                            h_sb[ch][:, i * 128:(i + 1) * 128],
                            w_sb[:, li * 128:(li + 1) * 128],
                            start=True, stop=True,
                        )
                    odt = dt.float32 if out_fp32 else dt.bfloat16
                    z_o = zout_pool.tile([128, C], odt,
                                         tag="zo" + ("f" if out_fp32 else "b"),
                                         name=f"z_o_{li}_{i}")
                    nc.scalar.activation(z_o[:], z_ps[:],
                                         mybir.ActivationFunctionType.Relu)
                    res.append(z_o)
                return res

            # ---- layer 1 ----
            z1_tiles = layer(0, out_fp32=False)
            for i in range(MT):
                nc.scalar.dma_start(out=z1_loc[i * 128:(i + 1) * 128, :],
                                    in_=z1_tiles[i][:])
            nc.gpsimd.collective_compute(
                "AllGather",
                mybir.AluOpType.bypass,
                replica_groups=[list(range(NCORES))],
                ins=[z1_loc.ap().opt()],
                outs=[z1_full.ap().opt()],
            )
            # reload the gathered Z1 into the (recycled) Z tiles
            for k in range(KCH):
                nc.scalar.dma_start(out=free3(z_sb[k], C), in_=z1f_v[k])

            # ---- layer 2 ----
            z2_tiles = layer(1, out_fp32=True)
            for i in range(MT):
                nc.sync.dma_start(out=out_d[i * 128:(i + 1) * 128, :],
                                  in_=z2_tiles[i][:])

    nc.compile()
    return nc


def _prep_inputs(x, net_params, A):
    a_bf = A.astype(BF16)
    z0 = np.ascontiguousarray(x.transpose(1, 0, 2).reshape(N, C)).astype(BF16)
    w = net_params.astype(np.float32).reshape(L, F, F).astype(BF16)
    # block-diagonal weight tile per layer: diag(W_l, W_l)
    w_sb = np.zeros((128, 2 * 128), dtype=BF16)
    for li in range(L):
        w_sb[0:F, li * 128:li * 128 + F] = w[li]
        w_sb[F:2 * F, li * 128 + F:li * 128 + 2 * F] = w[li]
    in_maps = []
    for j in range(NCORES):
        at_j = np.ascontiguousarray(a_bf[j * M_CORE:(j + 1) * M_CORE, :].T)
        in_maps.append({"at": at_j, "z0": z0, "w": w_sb})
    return in_maps


def kernel(x, t, net_params, A):
    x = np.asarray(x)
    A = np.asarray(A)
    net_params = np.asarray(net_params)

    if "nc" not in _CACHED:
        _CACHED["nc"] = _build_program()
    nc = _CACHED["nc"]

    in_maps = _prep_inputs(x, net_params, A)
    _CACHED["in_maps"] = in_maps
    res = run_bass_kernel_spmd(nc, in_maps, list(range(NCORES)))
    full = np.concatenate([res.results[c]["out"] for c in range(NCORES)], axis=0)
    return np.ascontiguousarray(full.reshape(N, B, F).transpose(1, 0, 2))


# revision 19
# speedup vs baseline: 1.1033x; 1.1033x over previous
"""Trainium2 Bass kernel for a 2-layer dense GCN block:

    z = x.reshape(B, N, F)                     # B=4, N=8192, F=64
    for i in range(2):
        z = relu((A @ z) @ W_i)                # A: [N, N] dense
    return z

Strategy (8 NeuronCores, SPMD):
  * Shard the output rows (m) of A @ Z across cores: core j owns rows
    [1024*j, 1024*(j+1)).  On the host we hand core j the matching
    column-slice of A^T (so the contraction dim n lands on SBUF
    partitions with a natural, contiguous DMA) cast to bf16 (16 MB —
    stays resident in SBUF for BOTH layers, so A is read from HBM once).
  * Z is handled as a [n, c] matrix with c = b*F + f (256 columns).
    Layer matmuls compute H^T[c, m] = sum_n Z[n, c] * A^T[n, m] on the
    tensor engine (lhsT = Z tile, rhs = A^T tile, fp32 PSUM accum).
    The n-loop is OUTERMOST so PE consumption tracks the chunked A/Z
    DMA arrival — no long PE-idle gaps, which keeps the HAM clock gate
    at full rate.
  * The per-layer weight apply uses a block-diagonal weight tile
    diag(W_i, W_i) so one K=128 matmul per c-half computes
    Z[m, c] = H^T.T @ diag(W, W) for two batches at once (all operands
    at base partition 0 — base-64 matmuls crash the runtime here).
  * Between layers, FOUR m-sliced AllGathers exchange each core's Z1
    slice; layer 2's n-loop is ordered by gather arrival so only the
    first slice's latency is exposed.
  * bf16 operands / fp32 accumulation throughout (measured ~0.5% rel-l2
    vs the fp32 reference).  Final output is fp32.
"""

import numpy as np
import ml_dtypes

import concourse.mybir as mybir
import concourse.tile as tile
from concourse import bacc
from concourse.bass_utils import run_bass_kernel_spmd

BF16 = ml_dtypes.bfloat16

NCORES = 8
B, N, F, L = 4, 8192, 64, 2
C = B * F                      # 256 columns of the Z matrix
M_CORE = N // NCORES           # 1024 output rows per core
NT = N // 128                  # 64 contraction tiles of 128
MT = M_CORE // 128             # 8 output-row tiles of 128 per core
KCH = 8                        # big DMA chunks for the resident A^T shard
TPC = NT // KCH                # 8 n-tiles per chunk
NG = 4                         # m-sliced inter-layer AllGathers
MPG = MT // NG                 # m-tiles per gather slice

_CACHED = {}


def _build_program():
    nc = bacc.Bacc("TRN2", target_bir_lowering=False, debug=False,
                   num_devices=NCORES)
    dt = mybir.dt

    at_d = nc.dram_tensor("at", [N, M_CORE], dt.bfloat16, kind="ExternalInput")
    z0_d = nc.dram_tensor("z0", [N, C], dt.bfloat16, kind="ExternalInput")
    w_d = nc.dram_tensor("w", [128, 2 * 128], dt.bfloat16, kind="ExternalInput")
    out_d = nc.dram_tensor("out", [M_CORE, C], dt.float32, kind="ExternalOutput")

    z1_loc = nc.dram_tensor("z1_loc", [M_CORE, C], dt.bfloat16)
    z1g = [nc.dram_tensor(f"z1g{g}", [NCORES * MPG * 128, C], dt.bfloat16)
           for g in range(NG)]

    # DRAM views with the n-tile index split out:  [chunk, p, local tile, cols]
    at_v = at_d.ap().rearrange("(k t p) m -> k p t m", k=KCH, p=128)
    z0_v = z0_d.ap().rearrange("(k t p) c -> k p t c", k=KCH, p=128)

    with tile.TileContext(nc) as tc:
        with tc.tile_pool(name="a_res", bufs=1) as a_pool, \
             tc.tile_pool(name="z_res", bufs=1) as z_pool, \
             tc.tile_pool(name="wk", bufs=1) as w_pool, \
             tc.tile_pool(name="ht", bufs=1, space="PSUM") as psh_pool, \
             tc.tile_pool(name="pz", bufs=2, space="PSUM") as psz_pool, \
             tc.tile_pool(name="hsb", bufs=2) as hsb_pool, \
             tc.tile_pool(name="zout", bufs=4) as zout_pool:

            w_sb = w_pool.tile([128, 2 * 128], dt.bfloat16, tag="w")
            nc.scalar.dma_start(out=w_sb[:], in_=w_d[:])

            # Resident A^T shard (16 MB bf16) + Z tiles, loaded in KCH big
            # chunks, interleaved so layer-1 matmuls can start early.
            at_sb = [a_pool.tile([128, TPC * M_CORE], dt.bfloat16,
                                 tag=f"at{k}", name=f"at_sb{k}")
                     for k in range(KCH)]
            z_sb = [z_pool.tile([128, TPC * C], dt.bfloat16,
                                tag=f"z{k}", name=f"z_sb{k}")
                    for k in range(KCH)]

            def free3(tile_ap, inner):
                return tile_ap.rearrange("p (t i) -> p t i", i=inner)

            for k in range(KCH):
                nc.scalar.dma_start(out=free3(z_sb[k][:], C), in_=z0_v[k])
                if k == 0:
                    # split the first chunk so the first matmuls start sooner
                    for q in range(4):
                        nc.sync.dma_start(
                            out=free3(at_sb[0][:, q * 2 * M_CORE:(q + 1) * 2 * M_CORE], M_CORE),
                            in_=at_v[0, :, q * 2:(q + 1) * 2, :])
                else:
                    nc.sync.dma_start(out=free3(at_sb[k][:], M_CORE), in_=at_v[k])

            def z_tile(t, ch):
                """lhsT: Z[n-tile t, c-half ch] -> [128, 128] bf16."""
                k, tt = divmod(t, TPC)
                return z_sb[k][:, tt * C + ch * 128: tt * C + ch * 128 + 128]

            def at_tile(t, mh):
                """rhs: A^T[n-tile t, m-half mh] -> [128, 512] bf16."""
                k, tt = divmod(t, TPC)
                return at_sb[k][:, tt * M_CORE + mh * 512: tt * M_CORE + mh * 512 + 512]

            def layer(li, t_order, out_fp32):
                # --- big matmul: H^T[c, m] accumulated over the 64 n-tiles,
                # n-loop outermost (t_order = arrival order) ---
                h_ps = {}
                for ch in range(2):
                    for mh in range(2):
                        h_ps[ch, mh] = psh_pool.tile([128, 512], dt.float32,
                                                     tag=f"hps{ch}{mh}",
                                                     name=f"hps_{li}_{ch}{mh}")
                for ti, t in enumerate(t_order):
                    for ch in range(2):
                        for mh in range(2):
                            nc.tensor.matmul(
                                h_ps[ch, mh][:],
                                z_tile(t, ch),
                                at_tile(t, mh),
                                start=(ti == 0),
                                stop=(ti == NT - 1),
                            )
                # --- PSUM -> SBUF (cast bf16) ---
                h_sb = [hsb_pool.tile([128, M_CORE], dt.bfloat16,
                                      tag=f"h{ch}", name=f"h_sb_{li}_{ch}")
                        for ch in range(2)]
                for ch in range(2):
                    for mh in range(2):
                        nc.vector.tensor_copy(
                            h_sb[ch][:, mh * 512:(mh + 1) * 512],
                            h_ps[ch, mh][:],
                        )
                # --- weight apply + relu: Z[m, c], one matmul per c-half
                # (lhsT = H^T chunk [c=128, m=128], rhs = diag(W, W)) ---
                res = []
                for i in range(MT):
                    z_ps = psz_pool.tile([128, C], dt.float32, tag="zps",
                                         name=f"z_ps_{li}_{i}")
                    for ch in range(2):
                        nc.tensor.matmul(
                            z_ps[:, ch * 128:(ch + 1) * 128],
                            h_sb[ch][:, i * 128:(i + 1) * 128],
                            w_sb[:, li * 128:(li + 1) * 128],
                            start=True, stop=True,
                        )
                    odt = dt.float32 if out_fp32 else dt.bfloat16
                    z_o = zout_pool.tile([128, C], odt,
                                         tag="zo" + ("f" if out_fp32 else "b"),
                                         name=f"z_o_{li}_{i}")
                    nc.scalar.activation(z_o[:], z_ps[:],
                                         mybir.ActivationFunctionType.Relu)
                    res.append(z_o)
                return res

            # ---- layer 1 (t in load order) ----
            z1_tiles = layer(0, list(range(NT)), out_fp32=False)
            for i in range(MT):
                nc.scalar.dma_start(out=z1_loc[i * 128:(i + 1) * 128, :],
                                    in_=z1_tiles[i][:])
            # ---- m-sliced AllGathers + reload into the recycled Z tiles ----
            for g in range(NG):
                r0, r1 = g * MPG * 128, (g + 1) * MPG * 128
                nc.gpsimd.collective_compute(
                    "AllGather",
                    mybir.AluOpType.bypass,
                    replica_groups=[list(range(NCORES))],
                    ins=[z1_loc.ap()[r0:r1, :].opt()],
                    outs=[z1g[g].ap().opt()],
                )
                rows = MPG * 128
                for k in range(KCH):
                    nc.scalar.dma_start(
                        out=free3(z_sb[k][:, MPG * g * C: MPG * (g + 1) * C], C),
                        in_=z1g[g].ap()[k * rows:(k + 1) * rows, :]
                            .rearrange("(t p) c -> p t c", p=128))

            # ---- layer 2 (t ordered by gather arrival) ----
            t2 = [8 * k + MPG * g + tt
                  for g in range(NG) for k in range(KCH) for tt in range(MPG)]
            z2_tiles = layer(1, t2, out_fp32=True)
            for i in range(MT):
                nc.sync.dma_start(out=out_d[i * 128:(i + 1) * 128, :],
                                  in_=z2_tiles[i][:])

    nc.compile()
    return nc


def _prep_inputs(x, net_params, A):
    a_bf = A.astype(BF16)
    z0 = np.ascontiguousarray(x.transpose(1, 0, 2).reshape(N, C)).astype(BF16)
    w = net_params.astype(np.float32).reshape(L, F, F).astype(BF16)
    # block-diagonal weight tile per layer: diag(W_l, W_l)
    w_sb = np.zeros((128, 2 * 128), dtype=BF16)
    for li in range(L):
        w_sb[0:F, li * 128:li * 128 + F] = w[li]
        w_sb[F:2 * F, li * 128 + F:li * 128 + 2 * F] = w[li]
    in_maps = []
    for j in range(NCORES):
        at_j = np.ascontiguousarray(a_bf[j * M_CORE:(j + 1) * M_CORE, :].T)
        in_maps.append({"at": at_j, "z0": z0, "w": w_sb})
    return in_maps


def kernel(x, t, net_params, A):
    x = np.asarray(x)
    A = np.asarray(A)
    net_params = np.asarray(net_params)

    if "nc" not in _CACHED:
        _CACHED["nc"] = _build_program()
    nc = _CACHED["nc"]

    in_maps = _prep_inputs(x, net_params, A)
    _CACHED["in_maps"] = in_maps
    res = run_bass_kernel_spmd(nc, in_maps, list(range(NCORES)))
    full = np.concatenate([res.results[c]["out"] for c in range(NCORES)], axis=0)
    return np.ascontiguousarray(full.reshape(N, B, F).transpose(1, 0, 2))


# revision 21
# speedup vs baseline: 1.2135x; 1.0999x over previous
"""Trainium2 Bass kernel for a 2-layer dense GCN block:

    z = x.reshape(B, N, F)                     # B=4, N=8192, F=64
    for i in range(2):
        z = relu((A @ z) @ W_i)                # A: [N, N] dense
    return z

Strategy (8 NeuronCores, SPMD):
  * Shard the output rows (m) of A @ Z across cores: core j owns rows
    [1024*j, 1024*(j+1)).  On the host we hand core j the matching
    column-slice of A^T (so the contraction dim n lands on SBUF
    partitions with a natural, contiguous DMA) cast to bf16 (16 MB —
    stays resident in SBUF for BOTH layers, so A is read from HBM once).
  * Z is handled as a [n, c] matrix with c = b*F + f (256 columns).
    Layer matmuls compute H^T[c, m] = sum_n Z[n, c] * A^T[n, m] on the
    tensor engine (lhsT = Z tile, rhs = A^T tile, fp32 PSUM accum).
    The n-loop is OUTERMOST so PE consumption tracks the chunked A/Z
    DMA arrival — no long PE-idle gaps, which keeps the HAM clock gate
    at full rate.
  * The per-layer weight apply uses a block-diagonal weight tile
    diag(W_i, W_i) so one K=128 matmul per c-half computes
    Z[m, c] = H^T.T @ diag(W, W) for two batches at once (all operands
    at base partition 0 — base-64 matmuls crash the runtime here).
  * Between layers, FOUR m-sliced AllGathers exchange each core's Z1
    slice; layer 2's n-loop is ordered by gather arrival so only the
    first slice's latency is exposed.
  * bf16 operands / fp32 accumulation throughout (measured ~0.5% rel-l2
    vs the fp32 reference).  Final output is fp32.
"""

import numpy as np
import ml_dtypes

import concourse.mybir as mybir
import concourse.tile as tile
from concourse import bacc
from concourse.bass_utils import run_bass_kernel_spmd

BF16 = ml_dtypes.bfloat16

NCORES = 8
B, N, F, L = 4, 8192, 64, 2
C = B * F                      # 256 columns of the Z matrix
M_CORE = N // NCORES           # 1024 output rows per core
NT = N // 128                  # 64 contraction tiles of 128
MT = M_CORE // 128             # 8 output-row tiles of 128 per core
KCH = 16                       # DMA chunks for the resident A^T shard
TPC = NT // KCH                # 8 n-tiles per chunk
NG = 4                         # m-sliced inter-layer AllGathers
MPG = MT // NG                 # m-tiles per gather slice

_CACHED = {}


def _build_program():
    nc = bacc.Bacc("TRN2", target_bir_lowering=False, debug=False,
                   num_devices=NCORES)
    dt = mybir.dt

    at_d = nc.dram_tensor("at", [N, M_CORE], dt.bfloat16, kind="ExternalInput")
    z0_d = nc.dram_tensor("z0", [N, C], dt.bfloat16, kind="ExternalInput")
    w_d = nc.dram_tensor("w", [128, 2 * 128], dt.bfloat16, kind="ExternalInput")
    out_d = nc.dram_tensor("out", [M_CORE, C], dt.float32, kind="ExternalOutput")

    z1_loc = nc.dram_tensor("z1_loc", [M_CORE, C], dt.bfloat16)
    warm_in = nc.dram_tensor("warm_in", [1, 128], dt.bfloat16)
    warm_out = nc.dram_tensor("warm_out", [NCORES, 128], dt.bfloat16)
    z1g = [nc.dram_tensor(f"z1g{g}", [NCORES * MPG * 128, C], dt.bfloat16)
           for g in range(NG)]

    # DRAM views with the n-tile index split out:  [chunk, p, local tile, cols]
    at_v = at_d.ap().rearrange("(k t p) m -> k p t m", k=KCH, p=128)
    z0_v = z0_d.ap().rearrange("(k t p) c -> k p t c", k=KCH, p=128)

    with tile.TileContext(nc) as tc:
        with tc.tile_pool(name="a_res", bufs=1) as a_pool, \
             tc.tile_pool(name="z_res", bufs=1) as z_pool, \
             tc.tile_pool(name="wk", bufs=1) as w_pool, \
             tc.tile_pool(name="ht", bufs=1, space="PSUM") as psh_pool, \
             tc.tile_pool(name="pz", bufs=2, space="PSUM") as psz_pool, \
             tc.tile_pool(name="hsb", bufs=2) as hsb_pool, \
             tc.tile_pool(name="zout", bufs=4) as zout_pool:

            w_sb = w_pool.tile([128, 2 * 128], dt.bfloat16, tag="w")
            nc.scalar.dma_start(out=w_sb[:], in_=w_d[:])

            # Resident A^T shard (16 MB bf16) + Z tiles, loaded in KCH big
            # chunks, interleaved so layer-1 matmuls can start early.
            at_sb = [a_pool.tile([128, TPC * M_CORE], dt.bfloat16,
                                 tag=f"at{k}", name=f"at_sb{k}")
                     for k in range(KCH)]
            z_sb = [z_pool.tile([128, TPC * C], dt.bfloat16,
                                tag=f"z{k}", name=f"z_sb{k}")
                    for k in range(KCH)]

            def free3(tile_ap, inner):
                return tile_ap.rearrange("p (t i) -> p t i", i=inner)

            for k in range(KCH):
                nc.scalar.dma_start(out=free3(z_sb[k][:], C), in_=z0_v[k])
                nc.sync.dma_start(out=free3(at_sb[k][:], M_CORE), in_=at_v[k])

            def z_tile(t, ch):
                """lhsT: Z[n-tile t, c-half ch] -> [128, 128] bf16."""
                k, tt = divmod(t, TPC)
                return z_sb[k][:, tt * C + ch * 128: tt * C + ch * 128 + 128]

            def at_tile(t, mh):
                """rhs: A^T[n-tile t, m-half mh] -> [128, 512] bf16."""
                k, tt = divmod(t, TPC)
                return at_sb[k][:, tt * M_CORE + mh * 512: tt * M_CORE + mh * 512 + 512]

            def layer(li, t_order, out_fp32):
                # --- big matmul: H^T[c, m] accumulated over the 64 n-tiles,
                # n-loop outermost (t_order = arrival order) ---
                h_ps = {}
                for ch in range(2):
                    for mh in range(2):
                        h_ps[ch, mh] = psh_pool.tile([128, 512], dt.float32,
                                                     tag=f"hps{ch}{mh}",
                                                     name=f"hps_{li}_{ch}{mh}")
                for ti, t in enumerate(t_order):
                    for ch in range(2):
                        for mh in range(2):
                            nc.tensor.matmul(
                                h_ps[ch, mh][:],
                                z_tile(t, ch),
                                at_tile(t, mh),
                                start=(ti == 0),
                                stop=(ti == NT - 1),
                            )
                # --- PSUM -> SBUF (cast bf16) ---
                h_sb = [hsb_pool.tile([128, M_CORE], dt.bfloat16,
                                      tag=f"h{ch}", name=f"h_sb_{li}_{ch}")
                        for ch in range(2)]
                for ch in range(2):
                    for mh in range(2):
                        nc.vector.tensor_copy(
                            h_sb[ch][:, mh * 512:(mh + 1) * 512],
                            h_ps[ch, mh][:],
                        )
                # --- weight apply + relu: Z[m, c], one matmul per c-half
                # (lhsT = H^T chunk [c=128, m=128], rhs = diag(W, W)) ---
                res = []
                for i in range(MT):
                    z_ps = psz_pool.tile([128, C], dt.float32, tag="zps",
                                         name=f"z_ps_{li}_{i}")
                    for ch in range(2):
                        nc.tensor.matmul(
                            z_ps[:, ch * 128:(ch + 1) * 128],
                            h_sb[ch][:, i * 128:(i + 1) * 128],
                            w_sb[:, li * 128:(li + 1) * 128],
                            start=True, stop=True,
                        )
                    odt = dt.float32 if out_fp32 else dt.bfloat16
                    z_o = zout_pool.tile([128, C], odt,
                                         tag="zo" + ("f" if out_fp32 else "b"),
                                         name=f"z_o_{li}_{i}")
                    nc.scalar.activation(z_o[:], z_ps[:],
                                         mybir.ActivationFunctionType.Relu)
                    res.append(z_o)
                return res

            # warm up the collective path early (hidden under layer 1)
            nc.gpsimd.dma_start(out=warm_in[:], in_=z0_d[0:1, 0:128])
            nc.gpsimd.collective_compute(
                "AllGather",
                mybir.AluOpType.bypass,
                replica_groups=[list(range(NCORES))],
                ins=[warm_in.ap().opt()],
                outs=[warm_out.ap().opt()],
            )

            # ---- layer 1 (t in load order) ----
            z1_tiles = layer(0, list(range(NT)), out_fp32=False)
            for i in range(MT):
                nc.scalar.dma_start(out=z1_loc[i * 128:(i + 1) * 128, :],
                                    in_=z1_tiles[i][:])
            # ---- m-sliced AllGathers + reload into the recycled Z tiles ----
            for g in range(NG):
                r0, r1 = g * MPG * 128, (g + 1) * MPG * 128
                nc.gpsimd.collective_compute(
                    "AllGather",
                    mybir.AluOpType.bypass,
                    replica_groups=[list(range(NCORES))],
                    ins=[z1_loc.ap()[r0:r1, :].opt()],
                    outs=[z1g[g].ap().opt()],
                )
                rows = MPG * 128
                for cb in range(NCORES):
                    t0g = 8 * cb + MPG * g          # first n-tile of this block
                    k, tto = divmod(t0g, TPC)
                    nc.scalar.dma_start(
                        out=free3(z_sb[k][:, tto * C:(tto + MPG) * C], C),
                        in_=z1g[g].ap()[cb * rows:(cb + 1) * rows, :]
                            .rearrange("(t p) c -> p t c", p=128))

            # ---- layer 2 (t ordered by gather arrival) ----
            t2 = [8 * cb + MPG * g + tt
                  for g in range(NG) for cb in range(NCORES) for tt in range(MPG)]
            z2_tiles = layer(1, t2, out_fp32=True)
            for i in range(MT):
                nc.sync.dma_start(out=out_d[i * 128:(i + 1) * 128, :],
                                  in_=z2_tiles[i][:])

    nc.compile()
    return nc


def _prep_inputs(x, net_params, A):
    a_bf = A.astype(BF16)
    z0 = np.ascontiguousarray(x.transpose(1, 0, 2).reshape(N, C)).astype(BF16)
    w = net_params.astype(np.float32).reshape(L, F, F).astype(BF16)
    # block-diagonal weight tile per layer: diag(W_l, W_l)
    w_sb = np.zeros((128, 2 * 128), dtype=BF16)
    for li in range(L):
        w_sb[0:F, li * 128:li * 128 + F] = w[li]
        w_sb[F:2 * F, li * 128 + F:li * 128 + 2 * F] = w[li]
    in_maps = []
    for j in range(NCORES):
        at_j = np.ascontiguousarray(a_bf[j * M_CORE:(j + 1) * M_CORE, :].T)
        in_maps.append({"at": at_j, "z0": z0, "w": w_sb})
    return in_maps


def kernel(x, t, net_params, A):
    x = np.asarray(x)
    A = np.asarray(A)
    net_params = np.asarray(net_params)

    if "nc" not in _CACHED:
        _CACHED["nc"] = _build_program()
    nc = _CACHED["nc"]

    in_maps = _prep_inputs(x, net_params, A)
    _CACHED["in_maps"] = in_maps
    res = run_bass_kernel_spmd(nc, in_maps, list(range(NCORES)))
    full = np.concatenate([res.results[c]["out"] for c in range(NCORES)], axis=0)
    return np.ascontiguousarray(full.reshape(N, B, F).transpose(1, 0, 2))
